# revision 9
# baseline (speedup 1.0000x reference)
import sys

for _p in ("/opt/trn_rl_repo", "/opt/trn_rl_repo/concourse"):
    if _p not in sys.path:
        sys.path.insert(0, _p)

import numpy as np
import ml_dtypes

N_CORES = 8
B, H, W_DIM, C = 8, 32, 32, 288
N = H * W_DIM          # 1024 points per core (batch-dim sharding: 1 image per core)
O = 64                 # codewords
CHUNK = 512            # PSUM bank free size (fp32)
NCH = N // CHUNK       # 2 chunks

# Per-engine cost (us) of one [<=128, 1024] abs-production tile-op, used to
# load-balance the static engine assignment. Act does 1-op Abs(x*1 - w) at
# 1.2 GHz; DVE does TS-sub (2x mode) + STT abs->fp8 (1x); Pool does the same
# two ops at 1.2 GHz / 0.6 sw efficiency.
COST_ACT = 1.07
COST_DVE = 1.81
COST_POOL = 3.10

_CACHE = {}


def _patch_drain_split():
    # The end-of-TileContext drain waits on the FULL global clock (engines +
    # one sem per DMA HW queue), overflowing the CTRL_NO struct's sync-wait
    # slots in walrus. Split: emit one 1-wait SP nop per clock component
    # first; the original drain's full-clock add_sem_waits then elides
    # everything via SP wait history.
    import concourse.tile as tile_mod
    from concourse.vector_clock import ScopedClock, VectorClock

    if getattr(tile_mod.TileContext, "_drain_split_patched", False):
        return

    def _drain_and_barrier(self, tick_clock, wait_clock):
        gc = tick_clock.global_clock
        for idx in range(len(gc)):
            tick = gc[idx]
            if tick <= 0:
                continue
            nop = self.nc.sync.nop(nofuse=True, hint="drain_split")
            vc = VectorClock()
            vc.require_at_least(idx, tick)
            wait_clock.add_sem_waits(nop.ins, ScopedClock({None: vc}))
        # Waitless drain: the nops above (same SP sequencer, in order)
        # already guarantee every sem is at its final value here.
        self.nc.sync.drain()
        self.nc.all_engine_barrier()
        assert self.sems is not None
        popped = self.nc._tile_sem_poison_stack.pop()
        assert popped is self._sem_poison
        self.nc.clear_and_free_semaphores(list(self.sems.allocated().values()))
        self.nc.all_engine_barrier()

    tile_mod.TileContext._drain_and_barrier = _drain_and_barrier
    tile_mod.TileContext._drain_split_patched = True


def _assign_units():
    """Static engine assignment for the 72 production units (64 full-o units
    + 8 tail-pair units), each 2 tile-ops, greedily balancing projected
    per-engine finish time."""
    units = [("full", o) for o in range(O)] + [("tail", gp) for gp in range(8)]
    # head-starts: DVE does Zdr memsets + absorbers; Pool/Act absorbers +
    # extraction work at the end.
    t = {"act": 1.5, "dve": 0.8, "pool": 1.9}
    cost = {"act": 2 * COST_ACT, "dve": 2 * COST_DVE, "pool": 2 * COST_POOL}
    out = []
    for u in units:
        e = min(t, key=lambda k: t[k] + cost[k])
        t[e] += cost[e]
        out.append((u, e))
    return out


def _build_program():
    import concourse.bass as bass
    import concourse.tile as tile
    from concourse import mybir

    _patch_drain_split()
    nc = bass.Bass("TRN2", debug=False, num_devices=N_CORES)

    f32 = mybir.dt.float32
    bf16 = mybir.dt.bfloat16
    fp8 = mybir.dt.float8e4
    Abs = mybir.ActivationFunctionType.Abs
    Ident = mybir.ActivationFunctionType.Identity
    DR = mybir.MatmulPerfMode.DoubleRow
    AOP = mybir.AluOpType

    # x transposed per core: rows = channel, cols = point. xa/xb are channel
    # blocks 0:128 / 128:256; xt is channels 256:288 replicated to all four
    # SBUF quadrants so one op covers the channel tail of four codewords.
    xa_d = nc.dram_tensor("xa", [128, N], bf16, kind="ExternalInput")
    xb_d = nc.dram_tensor("xb", [128, N], bf16, kind="ExternalInput")
    xt_d = nc.dram_tensor("xt", [128, N], bf16, kind="ExternalInput")
    # negW[p, 64*i + o] = -w[128*i + p, o] (bias operand for Act's Abs);
    # cols 128:144 hold the quadrant-packed tail: [32q+j, 128+g] =
    # -w[256+j, 4g+q]. Merged so the 8 dma_starts stay on distinct HW queues.
    negw_d = nc.dram_tensor("negw", [128, 2 * O + 16], f32, kind="ExternalInput")
    # +w variant (subtrahend for DVE/Pool tensor_scalar sub; TS scalar
    # operands must be fp32)
    wbf_d = nc.dram_tensor("wbf", [128, 2 * O + 16], f32, kind="ExternalInput")
    # tail routing one-hots: [32q+j, i, 64*gp + 4*(2gp+i)+q] = +1, and the
    # same pattern negated in cols 512:1024 (for Pool's min-tiles, whose
    # contribution enters the sum with weight -1).
    ztail_d = nc.dram_tensor("ztail", [128, 2, 16 * O], fp8, kind="ExternalInput")
    b_d = nc.dram_tensor("bvec", [O, 1], f32, kind="ExternalInput")
    out_d = nc.dram_tensor("out_t", [O, N], f32, kind="ExternalOutput")

    xa, xb, xt = xa_d.ap(), xb_d.ap(), xt_d.ap()
    negw, wbf = negw_d.ap(), wbf_d.ap()
    ztail_a, bvec, out_t = ztail_d.ap(), b_d.ap(), out_d.ap()

    from contextlib import ExitStack

    with tile.TileContext(nc) as tc, ExitStack() as ctx:
        const_pool = ctx.enter_context(tc.tile_pool(name="const", bufs=1))
        # One fresh buffer per production unit (72 x 256KB = 18.4MB SBUF):
        # reusing buffers would add WAW/WAR sem waits on the producing ops,
        # overflowing walrus's single sync-wait slot per instruction.
        prod_pool = ctx.enter_context(tc.tile_pool(name="prod", bufs=86))
        tmp_pool = ctx.enter_context(tc.tile_pool(name="tmp", bufs=1))
        psum_pool = ctx.enter_context(tc.tile_pool(name="ps", bufs=1, space="PSUM"))

        # --- SBUF constants -------------------------------------------------
        xa_sb = const_pool.tile([128, N], bf16, name="xa_sb")
        nc.sync.dma_start(xa_sb[:], xa[:, :])
        xb_sb = const_pool.tile([128, N], bf16, name="xb_sb")
        nc.sync.dma_start(xb_sb[:], xb[:, :])
        xt_sb = const_pool.tile([128, N], bf16, name="xt_sb")
        nc.sync.dma_start(xt_sb[:], xt[:, :])
        negw_sb = const_pool.tile([128, 2 * O + 16], f32, name="negw_sb")
        nc.sync.dma_start(negw_sb[:], negw[:, :])
        wbf_sb = const_pool.tile([128, 2 * O + 16], f32, name="wbf_sb")
        nc.sync.dma_start(wbf_sb[:], wbf[:, :])
        ztail_sb = const_pool.tile([128, 2, 16 * O], fp8, name="ztail_sb")
        nc.sync.dma_start(ztail_sb[:], ztail_a[:, :, :])
        b_sb = const_pool.tile([O, 1], f32, name="b_sb")
        nc.sync.dma_start(b_sb[:], bvec[:, :])

        # Full-pair routing: all-ones column at absolute col 63 (both k-subs);
        # lhsT slice [:, :, 63-o : 127-o] puts the hot column at local index o.
        # zdr_n is the -1 variant for Pool's min-tiles.
        zdr = const_pool.tile([128, 2, 128], fp8, name="zdr")
        nc.vector.memset(zdr[:], 0.0)
        nc.vector.memset(zdr[:, :, 63:64], 1.0)
        zdr_n = const_pool.tile([128, 2, 128], fp8, name="zdr_n")
        nc.vector.memset(zdr_n[:], 0.0)
        nc.vector.memset(zdr_n[:, :, 63:64], -1.0)

        # --- walrus 1-sync-wait discipline: per-engine absorber ops ---------
        # Each engine's first real op would otherwise need a fresh sem wait
        # per DMA queue it reads from. Absorb each input's DMA sem into the
        # engine's wait history with a cheap scratch op first.
        scr_d = const_pool.tile([1, 16], f32, name="scr_d")
        scr_p = const_pool.tile([1, 16], f32, name="scr_p")
        scr_a = const_pool.tile([1, 16], f32, name="scr_a")

        for k, src in enumerate((xa_sb, xb_sb, xt_sb, wbf_sb)):
            nc.vector.tensor_scalar_add(scr_d[0:1, k : k + 1], src[0:1, 0:1], 0.0)
            nc.gpsimd.tensor_scalar_add(scr_p[0:1, k : k + 1], src[0:1, 0:1], 0.0)
        # Act absorbers: first one touches only negw (bias+input from the same
        # tensor = one clock component), the rest reuse negw as bias.
        nc.scalar.activation(scr_a[0:1, 0:1], negw_sb[0:1, 0:1], Abs,
                             bias=negw_sb[0:1, 0:1])
        for k, src in enumerate((xa_sb, xb_sb, xt_sb, b_sb)):
            nc.scalar.activation(scr_a[0:1, k + 1 : k + 2], src[0:1, 0:1], Abs,
                                 bias=negw_sb[0:1, 0:1])

        # --- PSUM banks -----------------------------------------------------
        bank = [psum_pool.tile([128, CHUNK], f32, name=f"bank{ch}") for ch in range(NCH)]
        tbank = [psum_pool.tile([128, CHUNK], f32, name=f"tbank{ch}") for ch in range(NCH)]
        tinyb = psum_pool.tile([128, CHUNK], f32, name="tinyb")

        # PE absorbers: load the DVE (zdr memset) and ztail-DMA sems into PE
        # wait history via singleton matmuls before the real DR stream.
        nc.tensor.matmul(tinyb[0:1, 0:1], lhsT=zdr[:, 0, 0:1], rhs=zdr[:, 0, 0:1],
                         start=True, stop=True)
        nc.tensor.matmul(tinyb[0:1, 0:1], lhsT=ztail_sb[:, 0, 0:1],
                         rhs=ztail_sb[:, 0, 0:1], start=True, stop=True)

        tmp_d = tmp_pool.tile([128, N], bf16, name="tmp_d", tag="tmp_d")
        tmp_p = tmp_pool.tile([128, N], bf16, name="tmp_p", tag="tmp_p")

        def produce(eng, dst, dst_m, src, col, wpos, wneg):
            # dst <- |src - w| for act/dve. Pool (no 2-tensor-operand ops)
            # writes relu(src-w) to dst and min(src-w, 0) to dst_m; the
            # min-tiles are reduced with -1 routing weights.
            if eng == "act":
                nc.scalar.activation(dst, src, Abs,
                                     bias=wneg[:, col : col + 1])
            elif eng == "dve":
                nc.vector.tensor_scalar_sub(tmp_d[:], src,
                                            wpos[:, col : col + 1])
                nc.vector.scalar_tensor_tensor(dst, tmp_d[:], -1.0, tmp_d[:],
                                               op0=AOP.mult, op1=AOP.max)
            else:
                nc.gpsimd.tensor_scalar(dst, src, wpos[:, col : col + 1], 0.0,
                                        op0=AOP.subtract, op1=AOP.max)
                nc.gpsimd.tensor_scalar(dst_m, src, wpos[:, col : col + 1], 0.0,
                                        op0=AOP.subtract, op1=AOP.min)

        assignment = _assign_units()
        n_full_mm = sum(1 + (e == "pool") for (k, _), e in assignment if k == "full")
        full_done = [0]
        tail_mms = []

        def full_dr(o, tiles_with_sign):
            for lhsT, dt in tiles_with_sign:
                for ch in range(NCH):
                    nc.tensor.matmul(
                        bank[ch][0:O, :],
                        lhsT=lhsT[:, :, 63 - o : 127 - o],
                        rhs=dt[:, :, CHUNK * ch : CHUNK * (ch + 1)],
                        start=(full_done[0] == 0),
                        stop=(full_done[0] == n_full_mm - 1),
                        perf_mode=DR,
                    )
                full_done[0] += 1

        for (kind, a), eng in assignment:
            if kind == "full":
                o = a
                dt = prod_pool.tile([128, 2, N], fp8, name="dt", tag="u")
                dt_m = (prod_pool.tile([128, 2, N], fp8, name="dtm", tag="u")
                        if eng == "pool" else None)
                for i, src in enumerate((xa_sb, xb_sb)):
                    produce(eng, dt[:, i, :],
                            dt_m[:, i, :] if eng == "pool" else None,
                            src[:], i * O + o, wbf_sb, negw_sb)
                tiles = [(zdr, dt)] + ([(zdr_n, dt_m)] if eng == "pool" else [])
                full_dr(o, tiles)
            else:
                gp = a
                tt = prod_pool.tile([128, 2, N], fp8, name="tt", tag="u")
                tt_m = (prod_pool.tile([128, 2, N], fp8, name="ttm", tag="u")
                        if eng == "pool" else None)
                for i in range(2):
                    g = 2 * gp + i
                    produce(eng, tt[:, i, :],
                            tt_m[:, i, :] if eng == "pool" else None,
                            xt_sb[:], 2 * O + g, wbf_sb, negw_sb)
                tail_mms.append((gp, 0, tt))
                if eng == "pool":
                    tail_mms.append((gp, 8 * O, tt_m))

        # Tail DR matmuls last so the tail banks' stop ticks dominate the
        # full banks' in PE wait history at extraction time.
        for k, (gp, zoff, tt) in enumerate(tail_mms):
            for ch in range(NCH):
                nc.tensor.matmul(
                    tbank[ch][0:O, :],
                    lhsT=ztail_sb[:, :, zoff + O * gp : zoff + O * (gp + 1)],
                    rhs=tt[:, :, CHUNK * ch : CHUNK * (ch + 1)],
                    start=(k == 0),
                    stop=(k == len(tail_mms) - 1),
                    perf_mode=DR,
                )

        # --- extraction: out = bank + tbank + b -----------------------------
        # t1 on Act (Identity shares the loaded table with Abs); final add on
        # DVE (GPSIMD cannot access PSUM). The DVE absorber pre-loads the
        # full banks' PE stop tick so each STT carries only the Act wait.
        nc.vector.tensor_scalar_add(scr_d[0:1, 6:7], bank[1][0:1, 0:1], 0.0)
        out_sb = const_pool.tile([O, N], f32, name="out_sb")
        t1 = [const_pool.tile([O, CHUNK], f32, name=f"t1_{ch}") for ch in range(NCH)]
        for ch in range(NCH):
            nc.scalar.activation(t1[ch][:], tbank[ch][0:O, :], Ident,
                                 bias=b_sb[0:O, 0:1])
        for ch in range(NCH):
            nc.vector.scalar_tensor_tensor(
                out_sb[0:O, CHUNK * ch : CHUNK * (ch + 1)],
                t1[ch][:], 0.0, bank[ch][0:O, :],
                op0=AOP.add, op1=AOP.add,
            )

        nc.sync.dma_start(out_t[:, :], out_sb[:])

    return nc


def _prep_inputs(x, w, b):
    xs = x.reshape(B, N, C).astype(np.float32)
    w = np.asarray(w, dtype=np.float32)
    fp8 = ml_dtypes.float8_e4m3
    bf16 = ml_dtypes.bfloat16

    negw = np.zeros((128, 2 * O + 16), dtype=np.float32)
    for i in range(2):
        negw[:, i * O : (i + 1) * O] = -w[128 * i : 128 * (i + 1), :]
    for g in range(16):
        for q in range(4):
            negw[32 * q : 32 * q + 32, 2 * O + g] = -w[256:288, 4 * g + q]
    wbf = (-negw).astype(np.float32)

    ztail = np.zeros((128, 2, 16 * O), dtype=np.float32)
    for gp in range(8):
        for i in range(2):
            for q in range(4):
                o = 4 * (2 * gp + i) + q
                ztail[32 * q : 32 * q + 32, i, O * gp + o] = 1.0
                ztail[32 * q : 32 * q + 32, i, 8 * O + O * gp + o] = -1.0
    ztail = ztail.astype(fp8)

    bvec = np.asarray(b, dtype=np.float32).reshape(O, 1)

    in_maps = []
    for core in range(N_CORES):
        xT = xs[core].T  # [C, N]
        xa = xT[0:128].astype(bf16)
        xb = xT[128:256].astype(bf16)
        xt = np.tile(xT[256:288], (4, 1)).astype(bf16)
        in_maps.append({
            "xa": xa, "xb": xb, "xt": xt, "negw": negw, "wbf": wbf,
            "ztail": ztail, "bvec": bvec,
        })
    return in_maps


def kernel(x, w, b):
    from concourse.bass_utils import run_bass_kernel_spmd

    if "nc" not in _CACHE:
        _CACHE["nc"] = _build_program()
    nc = _CACHE["nc"]

    in_maps = _prep_inputs(x, w, b)
    res = run_bass_kernel_spmd(nc, in_maps, list(range(N_CORES)))
    out = np.stack(
        [np.asarray(res.results[core]["out_t"], dtype=np.float32).T for core in range(N_CORES)]
    )
    return out.astype(np.float32)


# revision 10
# speedup vs baseline: 8.0934x; 8.0934x over previous
import sys

for _p in ("/opt/trn_rl_repo", "/opt/trn_rl_repo/concourse"):
    if _p not in sys.path:
        sys.path.insert(0, _p)

import numpy as np
import ml_dtypes

N_CORES = 8
B, H, W_DIM, C = 8, 32, 32, 288
N = H * W_DIM          # 1024 points per core (batch-dim sharding: 1 image per core)
O = 64                 # codewords
CHUNK = 512            # PSUM bank free size (fp32)
NCH = N // CHUNK       # 2 chunks

# Per-engine cost (us) of one [<=128, 1024] abs-production tile-op, from HW
# trace: Act 1-op Abs(x*1 - w) = 1.36us; DVE TS-sub (2x mode, 0.58us) + STT
# abs->fp8 (1x, 1.47us) = 2.05us. GPSIMD is excluded: its software
# TensorScalar runs at ~18us per [128,1024] op AND slows co-running DVE ops
# to the same rate (measured lockstep poisoning).
COST_ACT = 1.36
COST_DVE = 2.05

_CACHE = {}


def _patch_drain_split():
    # The end-of-TileContext drain waits on the FULL global clock (engines +
    # one sem per DMA HW queue), overflowing the CTRL_NO struct's sync-wait
    # slots in walrus. Split: emit one 1-wait SP nop per clock component
    # first; the original drain's full-clock add_sem_waits then elides
    # everything via SP wait history.
    import concourse.tile as tile_mod
    from concourse.vector_clock import ScopedClock, VectorClock

    if getattr(tile_mod.TileContext, "_drain_split_patched", False):
        return

    def _drain_and_barrier(self, tick_clock, wait_clock):
        gc = tick_clock.global_clock
        for idx in range(len(gc)):
            tick = gc[idx]
            if tick <= 0:
                continue
            nop = self.nc.sync.nop(nofuse=True, hint="drain_split")
            vc = VectorClock()
            vc.require_at_least(idx, tick)
            wait_clock.add_sem_waits(nop.ins, ScopedClock({None: vc}))
        # Waitless drain: the nops above (same SP sequencer, in order)
        # already guarantee every sem is at its final value here.
        self.nc.sync.drain()
        self.nc.all_engine_barrier()
        assert self.sems is not None
        popped = self.nc._tile_sem_poison_stack.pop()
        assert popped is self._sem_poison
        self.nc.clear_and_free_semaphores(list(self.sems.allocated().values()))
        self.nc.all_engine_barrier()

    tile_mod.TileContext._drain_and_barrier = _drain_and_barrier
    tile_mod.TileContext._drain_split_patched = True


def _assign_units():
    """Static engine assignment for the 72 production units (64 full-o units
    + 8 tail-pair units), each 2 tile-ops, greedily balancing projected
    per-engine finish time."""
    units = [("full", o) for o in range(O)] + [("tail", gp) for gp in range(8)]
    # head-starts: DVE does Zdr memsets + absorbers + end extraction; Act
    # does absorbers + extraction t1s.
    t = {"act": 1.6, "dve": 2.6}
    cost = {"act": 2 * COST_ACT, "dve": 2 * COST_DVE}
    out = []
    for u in units:
        e = min(t, key=lambda k: t[k] + cost[k])
        t[e] += cost[e]
        out.append((u, e))
    return out


def _build_program():
    import concourse.bass as bass
    import concourse.tile as tile
    from concourse import mybir

    _patch_drain_split()
    nc = bass.Bass("TRN2", debug=False, num_devices=N_CORES)

    f32 = mybir.dt.float32
    bf16 = mybir.dt.bfloat16
    fp8 = mybir.dt.float8e4
    Abs = mybir.ActivationFunctionType.Abs
    Ident = mybir.ActivationFunctionType.Identity
    DR = mybir.MatmulPerfMode.DoubleRow
    AOP = mybir.AluOpType

    # x transposed per core: rows = channel, cols = point. xa/xb are channel
    # blocks 0:128 / 128:256; xt is channels 256:288 replicated to all four
    # SBUF quadrants so one op covers the channel tail of four codewords.
    xa_d = nc.dram_tensor("xa", [128, N], bf16, kind="ExternalInput")
    xb_d = nc.dram_tensor("xb", [128, N], bf16, kind="ExternalInput")
    xt_d = nc.dram_tensor("xt", [128, N], bf16, kind="ExternalInput")
    # negW[p, 64*i + o] = -w[128*i + p, o] (bias operand for Act's Abs);
    # cols 128:144 hold the quadrant-packed tail: [32q+j, 128+g] =
    # -w[256+j, 4g+q]. Merged so the 8 dma_starts stay on distinct HW queues.
    negw_d = nc.dram_tensor("negw", [128, 2 * O + 16], f32, kind="ExternalInput")
    # +w variant (subtrahend for DVE/Pool tensor_scalar sub; TS scalar
    # operands must be fp32)
    wbf_d = nc.dram_tensor("wbf", [128, 2 * O + 16], f32, kind="ExternalInput")
    # tail routing one-hots: [32q+j, i, 64*gp + 4*(2gp+i)+q] = +1, and the
    # same pattern negated in cols 512:1024 (for Pool's min-tiles, whose
    # contribution enters the sum with weight -1).
    ztail_d = nc.dram_tensor("ztail", [128, 2, 16 * O], fp8, kind="ExternalInput")
    b_d = nc.dram_tensor("bvec", [O, 1], f32, kind="ExternalInput")
    out_d = nc.dram_tensor("out_t", [O, N], f32, kind="ExternalOutput")

    xa, xb, xt = xa_d.ap(), xb_d.ap(), xt_d.ap()
    negw, wbf = negw_d.ap(), wbf_d.ap()
    ztail_a, bvec, out_t = ztail_d.ap(), b_d.ap(), out_d.ap()

    from contextlib import ExitStack

    with tile.TileContext(nc) as tc, ExitStack() as ctx:
        const_pool = ctx.enter_context(tc.tile_pool(name="const", bufs=1))
        # One fresh buffer per production unit (72 x 256KB = 18.4MB SBUF):
        # reusing buffers would add WAW/WAR sem waits on the producing ops,
        # overflowing walrus's single sync-wait slot per instruction.
        prod_pool = ctx.enter_context(tc.tile_pool(name="prod", bufs=72))
        tmp_pool = ctx.enter_context(tc.tile_pool(name="tmp", bufs=1))
        psum_pool = ctx.enter_context(tc.tile_pool(name="ps", bufs=1, space="PSUM"))

        # --- SBUF constants -------------------------------------------------
        xa_sb = const_pool.tile([128, N], bf16, name="xa_sb")
        nc.sync.dma_start(xa_sb[:], xa[:, :])
        xb_sb = const_pool.tile([128, N], bf16, name="xb_sb")
        nc.sync.dma_start(xb_sb[:], xb[:, :])
        xt_sb = const_pool.tile([128, N], bf16, name="xt_sb")
        nc.sync.dma_start(xt_sb[:], xt[:, :])
        negw_sb = const_pool.tile([128, 2 * O + 16], f32, name="negw_sb")
        nc.sync.dma_start(negw_sb[:], negw[:, :])
        wbf_sb = const_pool.tile([128, 2 * O + 16], f32, name="wbf_sb")
        nc.sync.dma_start(wbf_sb[:], wbf[:, :])
        ztail_sb = const_pool.tile([128, 2, 16 * O], fp8, name="ztail_sb")
        nc.sync.dma_start(ztail_sb[:], ztail_a[:, :, :])
        b_sb = const_pool.tile([O, 1], f32, name="b_sb")
        nc.sync.dma_start(b_sb[:], bvec[:, :])

        # Full-pair routing: all-ones column at absolute col 63 (both k-subs);
        # lhsT slice [:, :, 63-o : 127-o] puts the hot column at local index o.
        zdr = const_pool.tile([128, 2, 128], fp8, name="zdr")
        nc.vector.memset(zdr[:], 0.0)
        nc.vector.memset(zdr[:, :, 63:64], 1.0)

        # --- walrus 1-sync-wait discipline: per-engine absorber ops ---------
        # Each engine's first real op would otherwise need a fresh sem wait
        # per DMA queue it reads from. Absorb each input's DMA sem into the
        # engine's wait history with a cheap scratch op first.
        scr_d = const_pool.tile([1, 16], f32, name="scr_d")
        scr_a = const_pool.tile([1, 16], f32, name="scr_a")

        for k, src in enumerate((xa_sb, xb_sb, xt_sb, wbf_sb)):
            nc.vector.tensor_scalar_add(scr_d[0:1, k : k + 1], src[0:1, 0:1], 0.0)
        # Act absorbers: first one touches only negw (bias+input from the same
        # tensor = one clock component), the rest reuse negw as bias.
        nc.scalar.activation(scr_a[0:1, 0:1], negw_sb[0:1, 0:1], Abs,
                             bias=negw_sb[0:1, 0:1])
        for k, src in enumerate((xa_sb, xb_sb, xt_sb, b_sb)):
            nc.scalar.activation(scr_a[0:1, k + 1 : k + 2], src[0:1, 0:1], Abs,
                                 bias=negw_sb[0:1, 0:1])

        # --- PSUM banks -----------------------------------------------------
        bank = [psum_pool.tile([128, CHUNK], f32, name=f"bank{ch}") for ch in range(NCH)]
        tbank = [psum_pool.tile([128, CHUNK], f32, name=f"tbank{ch}") for ch in range(NCH)]
        tinyb = psum_pool.tile([128, CHUNK], f32, name="tinyb")

        # PE absorbers: load the DVE (zdr memset) and ztail-DMA sems into PE
        # wait history via singleton matmuls before the real DR stream.
        nc.tensor.matmul(tinyb[0:1, 0:1], lhsT=zdr[:, 0, 0:1], rhs=zdr[:, 0, 0:1],
                         start=True, stop=True)
        nc.tensor.matmul(tinyb[0:1, 0:1], lhsT=ztail_sb[:, 0, 0:1],
                         rhs=ztail_sb[:, 0, 0:1], start=True, stop=True)

        tmp_d = tmp_pool.tile([128, N], bf16, name="tmp_d", tag="tmp_d")

        def produce(eng, dst, src, col, wpos, wneg):
            if eng == "act":
                nc.scalar.activation(dst, src, Abs,
                                     bias=wneg[:, col : col + 1])
            else:
                nc.vector.tensor_scalar_sub(tmp_d[:], src,
                                            wpos[:, col : col + 1])
                nc.vector.scalar_tensor_tensor(dst, tmp_d[:], -1.0, tmp_d[:],
                                               op0=AOP.mult, op1=AOP.max)

        assignment = _assign_units()
        n_full = sum(1 for (k, _), _ in assignment if k == "full")
        full_done = [0]
        tail_mms = []

        for (kind, a), eng in assignment:
            if kind == "full":
                o = a
                dt = prod_pool.tile([128, 2, N], fp8, name="dt", tag="u")
                for i, src in enumerate((xa_sb, xb_sb)):
                    produce(eng, dt[:, i, :], src[:], i * O + o, wbf_sb, negw_sb)
                for ch in range(NCH):
                    nc.tensor.matmul(
                        bank[ch][0:O, :],
                        lhsT=zdr[:, :, 63 - o : 127 - o],
                        rhs=dt[:, :, CHUNK * ch : CHUNK * (ch + 1)],
                        start=(full_done[0] == 0),
                        stop=(full_done[0] == n_full - 1),
                        perf_mode=DR,
                    )
                full_done[0] += 1
            else:
                gp = a
                tt = prod_pool.tile([128, 2, N], fp8, name="tt", tag="u")
                for i in range(2):
                    g = 2 * gp + i
                    produce(eng, tt[:, i, :], xt_sb[:], 2 * O + g, wbf_sb, negw_sb)
                tail_mms.append((gp, 0, tt))

        # Tail DR matmuls last so the tail banks' stop ticks dominate the
        # full banks' in PE wait history at extraction time.
        for k, (gp, zoff, tt) in enumerate(tail_mms):
            for ch in range(NCH):
                nc.tensor.matmul(
                    tbank[ch][0:O, :],
                    lhsT=ztail_sb[:, :, zoff + O * gp : zoff + O * (gp + 1)],
                    rhs=tt[:, :, CHUNK * ch : CHUNK * (ch + 1)],
                    start=(k == 0),
                    stop=(k == len(tail_mms) - 1),
                    perf_mode=DR,
                )

        # --- extraction: out = bank + tbank + b -----------------------------
        # t1 on Act (Identity shares the loaded table with Abs); final add on
        # DVE (GPSIMD cannot access PSUM). The DVE absorber pre-loads the
        # full banks' PE stop tick so each STT carries only the Act wait.
        nc.vector.tensor_scalar_add(scr_d[0:1, 6:7], bank[1][0:1, 0:1], 0.0)
        out_sb = const_pool.tile([O, N], f32, name="out_sb")
        t1 = [const_pool.tile([O, CHUNK], f32, name=f"t1_{ch}") for ch in range(NCH)]
        for ch in range(NCH):
            nc.scalar.activation(t1[ch][:], tbank[ch][0:O, :], Ident,
                                 bias=b_sb[0:O, 0:1])
        for ch in range(NCH):
            nc.vector.scalar_tensor_tensor(
                out_sb[0:O, CHUNK * ch : CHUNK * (ch + 1)],
                t1[ch][:], 0.0, bank[ch][0:O, :],
                op0=AOP.add, op1=AOP.add,
            )

        nc.sync.dma_start(out_t[:, :], out_sb[:])

    return nc


def _prep_inputs(x, w, b):
    xs = x.reshape(B, N, C).astype(np.float32)
    w = np.asarray(w, dtype=np.float32)
    fp8 = ml_dtypes.float8_e4m3
    bf16 = ml_dtypes.bfloat16

    negw = np.zeros((128, 2 * O + 16), dtype=np.float32)
    for i in range(2):
        negw[:, i * O : (i + 1) * O] = -w[128 * i : 128 * (i + 1), :]
    for g in range(16):
        for q in range(4):
            negw[32 * q : 32 * q + 32, 2 * O + g] = -w[256:288, 4 * g + q]
    wbf = (-negw).astype(np.float32)

    ztail = np.zeros((128, 2, 16 * O), dtype=np.float32)
    for gp in range(8):
        for i in range(2):
            for q in range(4):
                o = 4 * (2 * gp + i) + q
                ztail[32 * q : 32 * q + 32, i, O * gp + o] = 1.0
                ztail[32 * q : 32 * q + 32, i, 8 * O + O * gp + o] = -1.0
    ztail = ztail.astype(fp8)

    bvec = np.asarray(b, dtype=np.float32).reshape(O, 1)

    in_maps = []
    for core in range(N_CORES):
        xT = xs[core].T  # [C, N]
        xa = xT[0:128].astype(bf16)
        xb = xT[128:256].astype(bf16)
        xt = np.tile(xT[256:288], (4, 1)).astype(bf16)
        in_maps.append({
            "xa": xa, "xb": xb, "xt": xt, "negw": negw, "wbf": wbf,
            "ztail": ztail, "bvec": bvec,
        })
    return in_maps


def kernel(x, w, b):
    from concourse.bass_utils import run_bass_kernel_spmd

    if "nc" not in _CACHE:
        _CACHE["nc"] = _build_program()
    nc = _CACHE["nc"]

    in_maps = _prep_inputs(x, w, b)
    res = run_bass_kernel_spmd(nc, in_maps, list(range(N_CORES)))
    out = np.stack(
        [np.asarray(res.results[core]["out_t"], dtype=np.float32).T for core in range(N_CORES)]
    )
    return out.astype(np.float32)
